# revision 22
# baseline (speedup 1.0000x reference)
"""GRU cell kernel for Trainium2, 8-core data-parallel.

All activations are staged feature-major ([128, B]) by the host so the device
streams them straight into the tensor engine (contraction over the 128-feature
partition dim). Per chunk, x and h arrive in ONE contiguous DMA (host
interleaves them per-chunk: [x_chunk | h_chunk]), biases ride in the weight
tensor, and gate biases are fused into the ScalarE activations.

Math (all bf16 operands, fp32 PSUM accumulate):
  z = sigmoid(Wz.T x + Uz.T h + bz)        2 matmuls -> PSUM, 1 ACT
  r = sigmoid(Wr.T x + Ur.T h + br)        2 matmuls, 1 ACT
  hc = tanh(Wh.T x + Uh.T (r*h) + bh)      2 matmuls, 1 ACT, 1 DVE mult
  h' = h + z*(hc - h)                      3 DVE ops
The Uh/tanh/blend stage is software-pipelined one chunk behind so the
in-order PE stream never waits on the DVE's r*h of the current chunk.

Engine budget per core (16384 batch cols): ACT ~48us (pacer), PE ~47us,
DVE ~43us, DMA ~36us. Head/tail chunks are tapered to shorten pipeline
fill/drain.
"""

from contextlib import ExitStack

import numpy as np

B = 131072
H = 128
NCORES = 8
BC = B // NCORES  # 16384 batch columns per core
CHUNK = 512  # matmul free dim (one PSUM bank)

CONFIG = {
    "free": 1024,  # ACT/DVE/DMA tile width
    "head": (512, 512),  # tapered leading chunk widths (pipeline fill)
    "tail": (512, 512),  # tapered trailing chunk widths (pipeline drain)
    "warmup": 14,  # dummy 128-col matmuls to ramp the PE p-state pre-data
    "tail_split": 4,  # last chunks: blend+store in 512-wide halves
    "io_bufs": 12,
    "mid_bufs": 8,
}

_CACHE = {}
LAST_RESULTS = None


def _chunks(cfg):
    FREE = cfg["free"]
    head = list(cfg.get("head") or ())
    tail = list(cfg.get("tail") or ())
    body = (BC - sum(head) - sum(tail)) // FREE
    widths = head + [FREE] * body + tail
    assert sum(widths) == BC, widths
    offs = np.cumsum([0] + widths[:-1]).tolist()
    return list(zip(offs, widths))


def _build_program(n_passes=1, mode="full", cfg=None):
    import concourse.bass as bass  # noqa: F401
    import concourse.tile as tile
    from concourse import bacc, mybir

    cfg = dict(CONFIG, **(cfg or {}))

    f32 = mybir.dt.float32
    bf16 = mybir.dt.bfloat16

    Sig = mybir.ActivationFunctionType.Sigmoid
    Tanh = mybir.ActivationFunctionType.Tanh
    Mult = mybir.AluOpType.mult
    Sub = mybir.AluOpType.subtract
    Add = mybir.AluOpType.add

    nc = bacc.Bacc(
        "TRN2",
        target_bir_lowering=False,
        debug=False,
        enable_asserts=False,
        num_devices=NCORES,
    )

    # Weight order on the stacked dim: Wz, Uz, Wr, Ur, Wh, Uh; 3 bias columns
    # (bz, br, bh) appended.
    Wz_i, Uz_i, Wr_i, Ur_i, Wh_i, Uh_i = range(6)
    WBB = nc.dram_tensor("WBB", [H, 6 * H + 3], bf16, kind="ExternalInput").ap()
    XHB = nc.dram_tensor("XHB", [H, 2 * BC], bf16, kind="ExternalInput").ap()
    OT = nc.dram_tensor("oT", [H, BC], bf16, kind="ExternalOutput").ap()

    FREE = cfg["free"]
    chunks = _chunks(cfg)

    with tile.TileContext(nc) as tc:
        with ExitStack() as ctx:
            consts = ctx.enter_context(tc.tile_pool(name="consts", bufs=1))
            io = ctx.enter_context(tc.tile_pool(name="io", bufs=cfg["io_bufs"]))
            mid = ctx.enter_context(tc.tile_pool(name="mid", bufs=cfg["mid_bufs"]))
            psum = ctx.enter_context(tc.tile_pool(name="psum", bufs=2, space="PSUM"))

            wbb = consts.tile([H, 6 * H + 3], bf16)
            nc.sync.dma_start(wbb[:], WBB)
            wm = [wbb[:, i * H : (i + 1) * H] for i in range(6)]
            bz, br, bh = (wbb[:, 6 * H + i : 6 * H + i + 1] for i in range(3))

            carry = None

            # PE p-state warmup: dummy matmuls on a zeroed SBUF tile keep the
            # tensor engine continuously busy from program start, so the real
            # matmuls hit the max clock immediately instead of ramping
            # 1.4->2.4GHz across the first chunks. Runs while the first data
            # DMAs are in flight; writes a scratch region the first real
            # accumulation group resets (start=True).
            n_warm = cfg.get("warmup") or 0
            if mode == "full" and n_warm:
                ones = mid.tile([H, H], bf16, tag="warm")
                nc.vector.memset(ones[:], 0.0)
                pwarm = psum.tile([H, FREE], f32, tag="pz", bufs=1)
                for _ in range(n_warm):
                    nc.tensor.matmul(pwarm[:, 0:H], ones[:], ones[:],
                                     start=True, stop=True)

            def emit_tail_mm(s):
                # Uh accumulation closes the carried chunk's h-candidate PSUM.
                for ss in s["mm"]:
                    nc.tensor.matmul(
                        s["ph"][:, ss], wm[Uh_i], s["rh"][:, ss],
                        start=False, stop=True,
                    )

            def emit_tail(s, split=False):
                # tanh + blend + store for the carried chunk. With split=True
                # the blend+store run in 512-halves so the final out-DMAs
                # overlap the remaining blend work during pipeline drain.
                w = s["w"]
                hc = mid.tile([H, FREE], bf16, tag="hc")
                nc.scalar.activation(hc[:, :w], s["ph"][:, :w], Tanh, bias=bh)
                parts = s["mm"] if split else [slice(0, w)]
                for ps in parts:
                    d = mid.tile([H, FREE], bf16, tag="d")
                    nc.vector.tensor_tensor(d[:, ps], hc[:, ps], s["hs"][:, ps], Sub)
                    m = mid.tile([H, FREE], bf16, tag="m")
                    nc.vector.tensor_tensor(m[:, ps], s["z"][:, ps], d[:, ps], Mult)
                    o = mid.tile([H, FREE], bf16, tag="o")
                    nc.vector.tensor_tensor(o[:, ps], s["hs"][:, ps], m[:, ps], Add)
                    nc.sync.dma_start(
                        OT[:, s["sl"].start + ps.start : s["sl"].start + ps.stop],
                        o[:, ps])

            all_chunks = chunks * n_passes
            n_split = cfg.get("tail_split") or 0

            def want_split(s):
                return s["idx"] >= len(all_chunks) - n_split

            for idx, (off, w) in enumerate(all_chunks):
                sl = slice(off, off + w)
                mm = [slice(s, s + min(CHUNK, w - s)) for s in range(0, w, CHUNK)]
                xh = io.tile([H, 2 * FREE], bf16, tag="xh")
                nc.sync.dma_start(xh[:, : 2 * w], XHB[:, 2 * off : 2 * off + 2 * w])
                xs = xh[:, 0:w]
                hs = xh[:, w : 2 * w]

                if mode == "dma":
                    o = mid.tile([H, FREE], bf16, tag="o")
                    nc.vector.tensor_copy(o[:, :w], hs)
                    nc.sync.dma_start(OT[:, sl], o[:, :w])
                    continue

                pz = psum.tile([H, FREE], f32, tag="pz", bufs=1)
                pr = psum.tile([H, FREE], f32, tag="pr", bufs=1)
                ph = psum.tile([H, FREE], f32, tag="ph", bufs=2)

                # Grouped by weight matrix (not by slice) so the PE reloads
                # weights fewer times per chunk; the carried chunk's Uh mms
                # go right after Uz so its tanh is ready before ACT needs it.
                for ss in mm:
                    nc.tensor.matmul(pz[:, ss], wm[Wz_i], xs[:, ss],
                                     start=True, stop=False)
                for ss in mm:
                    nc.tensor.matmul(pz[:, ss], wm[Uz_i], hs[:, ss],
                                     start=False, stop=True)
                for ss in mm:
                    nc.tensor.matmul(pr[:, ss], wm[Wr_i], xs[:, ss],
                                     start=True, stop=False)
                for ss in mm:
                    nc.tensor.matmul(pr[:, ss], wm[Ur_i], hs[:, ss],
                                     start=False, stop=True)
                for ss in mm:
                    nc.tensor.matmul(ph[:, ss], wm[Wh_i], xs[:, ss],
                                     start=True, stop=False)

                z = mid.tile([H, FREE], bf16, tag="z")
                nc.scalar.activation(z[:, :w], pz[:, :w], Sig, bias=bz)
                r = mid.tile([H, FREE], bf16, tag="r")
                nc.scalar.activation(r[:, :w], pr[:, :w], Sig, bias=br)

                rh = mid.tile([H, FREE], bf16, tag="rh")
                nc.vector.tensor_tensor(rh[:, :w], r[:, :w], hs, Mult)

                if carry is not None:
                    emit_tail_mm(carry)
                    emit_tail(carry, split=want_split(carry))
                carry = dict(ph=ph, rh=rh, z=z, hs=hs, sl=sl, w=w, mm=mm,
                             idx=idx)
            if carry is not None:
                emit_tail_mm(carry)
                emit_tail(carry, split=want_split(carry))

    nc.compile()
    return nc


def _get_program(n_passes=1, mode="full", cfg=None):
    def freeze(v):
        return tuple(v) if isinstance(v, (list, tuple)) else v

    key = (n_passes, mode,
           tuple(sorted((k, freeze(v)) for k, v in (cfg or CONFIG).items())))
    if key not in _CACHE:
        _CACHE[key] = _build_program(n_passes, mode, cfg)
    return _CACHE[key]


def make_in_maps(x_t, h_prev, Wz, Uz, bz, Wr, Ur, br, Wh, Uh, bh, cfg=None):
    import ml_dtypes

    cfg = dict(CONFIG, **(cfg or {}))
    bf = ml_dtypes.bfloat16

    wbb = np.empty((H, 6 * H + 3), dtype=bf)
    for i, w in enumerate((Wz, Uz, Wr, Ur, Wh, Uh)):
        wbb[:, i * H : (i + 1) * H] = np.asarray(w, dtype=np.float32).astype(bf)
    for i, b in enumerate((bz, br, bh)):
        wbb[:, 6 * H + i] = np.asarray(b, dtype=np.float32).astype(bf)

    xT = np.asarray(x_t, dtype=np.float32).T.astype(bf)
    hT = np.asarray(h_prev, dtype=np.float32).T.astype(bf)

    chunks = _chunks(cfg)
    in_maps = []
    for c in range(NCORES):
        s0 = c * BC
        xhb = np.empty((H, 2 * BC), dtype=bf)
        for off, w in chunks:
            xhb[:, 2 * off : 2 * off + w] = xT[:, s0 + off : s0 + off + w]
            xhb[:, 2 * off + w : 2 * off + 2 * w] = hT[:, s0 + off : s0 + off + w]
        in_maps.append({"XHB": xhb, "WBB": wbb})
    return in_maps


def kernel(x_t, h_prev, Wz, Uz, bz, Wr, Ur, br, Wh, Uh, bh):
    global LAST_RESULTS
    from concourse import bass_utils

    in_maps = make_in_maps(x_t, h_prev, Wz, Uz, bz, Wr, Ur, br, Wh, Uh, bh)
    nc = _get_program()
    res = bass_utils.run_bass_kernel_spmd(nc, in_maps, core_ids=list(range(NCORES)))
    LAST_RESULTS = res

    oT = np.concatenate([r["oT"] for r in res.results], axis=1)  # [H, B]
    return np.ascontiguousarray(oT.T.astype(np.float32))


# revision 23
# speedup vs baseline: 1.0218x; 1.0218x over previous
"""GRU cell kernel for Trainium2, 8-core data-parallel.

All activations are staged feature-major ([128, B]) by the host so the device
streams them straight into the tensor engine (contraction over the 128-feature
partition dim). Per chunk, x and h arrive in ONE contiguous DMA (host
interleaves them per-chunk: [x_chunk | h_chunk]), biases ride in the weight
tensor, and gate biases are fused into the ScalarE activations.

Math (all bf16 operands, fp32 PSUM accumulate):
  z = sigmoid(Wz.T x + Uz.T h + bz)        2 matmuls -> PSUM, 1 ACT
  r = sigmoid(Wr.T x + Ur.T h + br)        2 matmuls, 1 ACT
  hc = tanh(Wh.T x + Uh.T (r*h) + bh)      2 matmuls, 1 ACT, 1 DVE mult
  h' = h + z*(hc - h)                      3 DVE ops
The Uh/tanh/blend stage is software-pipelined one chunk behind so the
in-order PE stream never waits on the DVE's r*h of the current chunk.

Engine budget per core (16384 batch cols): ACT ~48us (pacer), PE ~47us,
DVE ~43us, DMA ~36us. Head/tail chunks are tapered to shorten pipeline
fill/drain.
"""

from contextlib import ExitStack

import numpy as np

B = 131072
H = 128
NCORES = 8
BC = B // NCORES  # 16384 batch columns per core
CHUNK = 512  # matmul free dim (one PSUM bank)

CONFIG = {
    "free": 1024,  # ACT/DVE/DMA tile width
    "head": (512,),  # tapered leading chunk widths (pipeline fill)
    "tail": (512,),  # tapered trailing chunk widths (pipeline drain)
    "warmup": 30,  # dummy 128-col matmuls to ramp the PE p-state pre-data
    "tail_split": 3,  # last chunks: blend+store in 512-wide halves
    "io_bufs": 10,
    "mid_bufs": 6,
}

_CACHE = {}
LAST_RESULTS = None


def _chunks(cfg):
    FREE = cfg["free"]
    head = list(cfg.get("head") or ())
    tail = list(cfg.get("tail") or ())
    body = (BC - sum(head) - sum(tail)) // FREE
    widths = head + [FREE] * body + tail
    assert sum(widths) == BC, widths
    offs = np.cumsum([0] + widths[:-1]).tolist()
    return list(zip(offs, widths))


def _build_program(n_passes=1, mode="full", cfg=None):
    import concourse.bass as bass  # noqa: F401
    import concourse.tile as tile
    from concourse import bacc, mybir

    cfg = dict(CONFIG, **(cfg or {}))

    f32 = mybir.dt.float32
    bf16 = mybir.dt.bfloat16

    Sig = mybir.ActivationFunctionType.Sigmoid
    Tanh = mybir.ActivationFunctionType.Tanh
    Mult = mybir.AluOpType.mult
    Sub = mybir.AluOpType.subtract
    Add = mybir.AluOpType.add

    nc = bacc.Bacc(
        "TRN2",
        target_bir_lowering=False,
        debug=False,
        enable_asserts=False,
        num_devices=NCORES,
    )

    # Weight order on the stacked dim: Wz, Uz, Wr, Ur, Wh, Uh; 3 bias columns
    # (bz, br, bh) appended.
    Wz_i, Uz_i, Wr_i, Ur_i, Wh_i, Uh_i = range(6)
    WBB = nc.dram_tensor("WBB", [H, 6 * H + 3], bf16, kind="ExternalInput").ap()
    XHB = nc.dram_tensor("XHB", [H, 2 * BC], bf16, kind="ExternalInput").ap()
    OT = nc.dram_tensor("oT", [H, BC], bf16, kind="ExternalOutput").ap()

    FREE = cfg["free"]
    chunks = _chunks(cfg)

    with tile.TileContext(nc) as tc:
        with ExitStack() as ctx:
            consts = ctx.enter_context(tc.tile_pool(name="consts", bufs=1))
            io = ctx.enter_context(tc.tile_pool(name="io", bufs=cfg["io_bufs"]))
            mid = ctx.enter_context(tc.tile_pool(name="mid", bufs=cfg["mid_bufs"]))
            psum = ctx.enter_context(tc.tile_pool(name="psum", bufs=2, space="PSUM"))

            wbb = consts.tile([H, 6 * H + 3], bf16)
            nc.sync.dma_start(wbb[:], WBB)
            wm = [wbb[:, i * H : (i + 1) * H] for i in range(6)]
            bz, br, bh = (wbb[:, 6 * H + i : 6 * H + i + 1] for i in range(3))

            carry = None

            # PE p-state warmup: dummy matmuls on a zeroed SBUF tile keep the
            # tensor engine continuously busy from program start, so the real
            # matmuls hit the max clock immediately instead of ramping
            # 1.4->2.4GHz across the first chunks. Runs while the first data
            # DMAs are in flight; writes a scratch region the first real
            # accumulation group resets (start=True).
            n_warm = cfg.get("warmup") or 0
            if mode == "full" and n_warm:
                ones = mid.tile([H, H], bf16, tag="warm")
                nc.vector.memset(ones[:], 0.0)
                pwarm = psum.tile([H, FREE], f32, tag="pz", bufs=1)
                for _ in range(n_warm):
                    nc.tensor.matmul(pwarm[:, 0:H], ones[:], ones[:],
                                     start=True, stop=True)

            def emit_tail_mm(s):
                # Uh accumulation closes the carried chunk's h-candidate PSUM.
                for ss in s["mm"]:
                    nc.tensor.matmul(
                        s["ph"][:, ss], wm[Uh_i], s["rh"][:, ss],
                        start=False, stop=True,
                    )

            def emit_tail(s, split=False):
                # tanh + blend + store for the carried chunk. With split=True
                # the blend+store run in 512-halves so the final out-DMAs
                # overlap the remaining blend work during pipeline drain.
                w = s["w"]
                hc = mid.tile([H, FREE], bf16, tag="hc")
                nc.scalar.activation(hc[:, :w], s["ph"][:, :w], Tanh, bias=bh)
                parts = s["mm"] if split else [slice(0, w)]
                for ps in parts:
                    d = mid.tile([H, FREE], bf16, tag="d")
                    nc.vector.tensor_tensor(d[:, ps], hc[:, ps], s["hs"][:, ps], Sub)
                    m = mid.tile([H, FREE], bf16, tag="m")
                    nc.vector.tensor_tensor(m[:, ps], s["z"][:, ps], d[:, ps], Mult)
                    o = mid.tile([H, FREE], bf16, tag="o")
                    nc.vector.tensor_tensor(o[:, ps], s["hs"][:, ps], m[:, ps], Add)
                    nc.sync.dma_start(
                        OT[:, s["sl"].start + ps.start : s["sl"].start + ps.stop],
                        o[:, ps])

            all_chunks = chunks * n_passes
            n_split = cfg.get("tail_split") or 0

            def want_split(s):
                return s["idx"] >= len(all_chunks) - n_split

            for idx, (off, w) in enumerate(all_chunks):
                sl = slice(off, off + w)
                mm = [slice(s, s + min(CHUNK, w - s)) for s in range(0, w, CHUNK)]
                xh = io.tile([H, 2 * FREE], bf16, tag="xh")
                nc.sync.dma_start(xh[:, : 2 * w], XHB[:, 2 * off : 2 * off + 2 * w])
                xs = xh[:, 0:w]
                hs = xh[:, w : 2 * w]

                if mode == "dma":
                    o = mid.tile([H, FREE], bf16, tag="o")
                    nc.vector.tensor_copy(o[:, :w], hs)
                    nc.sync.dma_start(OT[:, sl], o[:, :w])
                    continue

                pz = psum.tile([H, FREE], f32, tag="pz", bufs=1)
                pr = psum.tile([H, FREE], f32, tag="pr", bufs=1)
                ph = psum.tile([H, FREE], f32, tag="ph", bufs=2)

                # Grouped by weight matrix (not by slice) so the PE reloads
                # weights fewer times per chunk; the carried chunk's Uh mms
                # go right after Uz so its tanh is ready before ACT needs it.
                for ss in mm:
                    nc.tensor.matmul(pz[:, ss], wm[Wz_i], xs[:, ss],
                                     start=True, stop=False)
                for ss in mm:
                    nc.tensor.matmul(pz[:, ss], wm[Uz_i], hs[:, ss],
                                     start=False, stop=True)
                for ss in mm:
                    nc.tensor.matmul(pr[:, ss], wm[Wr_i], xs[:, ss],
                                     start=True, stop=False)
                for ss in mm:
                    nc.tensor.matmul(pr[:, ss], wm[Ur_i], hs[:, ss],
                                     start=False, stop=True)
                for ss in mm:
                    nc.tensor.matmul(ph[:, ss], wm[Wh_i], xs[:, ss],
                                     start=True, stop=False)

                z = mid.tile([H, FREE], bf16, tag="z")
                nc.scalar.activation(z[:, :w], pz[:, :w], Sig, bias=bz)
                r = mid.tile([H, FREE], bf16, tag="r")
                nc.scalar.activation(r[:, :w], pr[:, :w], Sig, bias=br)

                rh = mid.tile([H, FREE], bf16, tag="rh")
                nc.vector.tensor_tensor(rh[:, :w], r[:, :w], hs, Mult)

                if carry is not None:
                    emit_tail_mm(carry)
                    emit_tail(carry, split=want_split(carry))
                carry = dict(ph=ph, rh=rh, z=z, hs=hs, sl=sl, w=w, mm=mm,
                             idx=idx)
            if carry is not None:
                emit_tail_mm(carry)
                emit_tail(carry, split=want_split(carry))

    nc.compile()
    return nc


def _get_program(n_passes=1, mode="full", cfg=None):
    def freeze(v):
        return tuple(v) if isinstance(v, (list, tuple)) else v

    key = (n_passes, mode,
           tuple(sorted((k, freeze(v)) for k, v in (cfg or CONFIG).items())))
    if key not in _CACHE:
        _CACHE[key] = _build_program(n_passes, mode, cfg)
    return _CACHE[key]


def make_in_maps(x_t, h_prev, Wz, Uz, bz, Wr, Ur, br, Wh, Uh, bh, cfg=None):
    import ml_dtypes

    cfg = dict(CONFIG, **(cfg or {}))
    bf = ml_dtypes.bfloat16

    wbb = np.empty((H, 6 * H + 3), dtype=bf)
    for i, w in enumerate((Wz, Uz, Wr, Ur, Wh, Uh)):
        wbb[:, i * H : (i + 1) * H] = np.asarray(w, dtype=np.float32).astype(bf)
    for i, b in enumerate((bz, br, bh)):
        wbb[:, 6 * H + i] = np.asarray(b, dtype=np.float32).astype(bf)

    xT = np.asarray(x_t, dtype=np.float32).T.astype(bf)
    hT = np.asarray(h_prev, dtype=np.float32).T.astype(bf)

    chunks = _chunks(cfg)
    in_maps = []
    for c in range(NCORES):
        s0 = c * BC
        xhb = np.empty((H, 2 * BC), dtype=bf)
        for off, w in chunks:
            xhb[:, 2 * off : 2 * off + w] = xT[:, s0 + off : s0 + off + w]
            xhb[:, 2 * off + w : 2 * off + 2 * w] = hT[:, s0 + off : s0 + off + w]
        in_maps.append({"XHB": xhb, "WBB": wbb})
    return in_maps


def kernel(x_t, h_prev, Wz, Uz, bz, Wr, Ur, br, Wh, Uh, bh):
    global LAST_RESULTS
    from concourse import bass_utils

    in_maps = make_in_maps(x_t, h_prev, Wz, Uz, bz, Wr, Ur, br, Wh, Uh, bh)
    nc = _get_program()
    res = bass_utils.run_bass_kernel_spmd(nc, in_maps, core_ids=list(range(NCORES)))
    LAST_RESULTS = res

    oT = np.concatenate([r["oT"] for r in res.results], axis=1)  # [H, B]
    return np.ascontiguousarray(oT.T.astype(np.float32))
